# revision 15
# baseline (speedup 1.0000x reference)
"""LRMC (masked low-rank matrix completion) kernel for 8 trn2 NeuronCores.

Algorithm (3 iterations):
    E = mask * (x - U V)        [d1, d2]
    U = U + E V^T               [d1, r]
    V = V + U^T E               [r, d2]   (updated U; V-update all-reduced)
out = U V.

Sharding: rows of x/mask/U over 8 cores (512 rows each); V replicated;
U^T E contribution all-reduced (bf16, split into two column halves so the
next iteration's column groups can start while the second half is in
flight) each iteration.

On-chip layouts (per core, m = 512 local rows, r = 128, d2 = 4096):
    mxT, maskT : [128p, 32nb, 512m]  bf16   (transposed masked-x / mask)
    e_T groups : [128p(d2), 1024]    bf16   (rotating, 2 nb blocks each)
    e_nat      : [128p(m), 4mb, 4096d2] bf16  (via DMA xbar transpose)
    U stored as U_T f32/bf16 [128r, 512m] + U_nat bf16 [128p(m), 4mb, 128r]
    V stored as V_nat bf16 [128r, 4096d2] + V_T bf16 [128p(d2), 32nb, 128r]

Matmuls (bf16 operands, fp32 PSUM):
    P_T[nb]   = matmul(lhsT=V_nat[:, nb*128:+128], rhs=U_T)          (M1)
    Udelta_T += matmul(lhsT=V_T[:, nb], rhs=e_T[nb])                 (M2)
    G[chunk] += matmul(lhsT=U_nat[:, mb], rhs=e_nat[:, mb, chunk])   (M3)
"""

import numpy as np
import ml_dtypes

import concourse.bass as bass
import concourse.tile as tile
from concourse import bacc, mybir
from concourse.bass_utils import run_bass_kernel_spmd

D1, D2, RANK, ITERS = 4096, 4096, 128, 3
N_CORES = 8
M = D1 // N_CORES          # 512 rows per core
MB = M // 128              # 4 row blocks
NB = D2 // 128             # 32 d2 blocks
NG = 16                    # groups of 2 nb blocks (1024 cols of P_T slab)
HALF = D2 // 2             # 2048
N_TICKS = 56               # PE keep-warm matmuls per AllReduce window

BF16 = mybir.dt.bfloat16
F32 = mybir.dt.float32
bf16 = ml_dtypes.bfloat16

_cache = {}


def _build():
    nc = bacc.Bacc("TRN2", target_bir_lowering=False, debug=False,
                   num_devices=N_CORES)

    mxT_d = nc.dram_tensor("mxT", [128, NB, M], BF16, kind="ExternalInput")
    mskT_d = nc.dram_tensor("mskT", [128, NB, M], BF16, kind="ExternalInput")
    uT32_d = nc.dram_tensor("uT32", [128, M], F32, kind="ExternalInput")
    unat_d = nc.dram_tensor("unat", [128, MB, 128], BF16, kind="ExternalInput")
    vb_d = nc.dram_tensor("vb", [128, D2], BF16, kind="ExternalInput")
    vT_d = nc.dram_tensor("vT", [128, NB, 128], BF16, kind="ExternalInput")
    out_d = nc.dram_tensor("out", [M, D2], F32, kind="ExternalOutput")

    with tile.TileContext(nc) as tc:
        with (
            tc.tile_pool(name="state", bufs=1) as state,
            tc.tile_pool(name="ustate", bufs=2) as ustate,
            tc.tile_pool(name="enat", bufs=1) as enatp,
            tc.tile_pool(name="rot", bufs=3) as rot,
            tc.tile_pool(name="et", bufs=4) as etp,
            tc.tile_pool(name="gout", bufs=1) as goutp,
            tc.tile_pool(name="dvp", bufs=1) as dvp,
            tc.tile_pool(name="ostage", bufs=3) as ostage,
            tc.tile_pool(name="psP", bufs=3, space="PSUM") as psP,
            tc.tile_pool(name="psU", bufs=1, space="PSUM") as psU,
            tc.tile_pool(name="dram", bufs=2, space="DRAM") as dram,
        ):
            # ---- load state (small tensors first so matmuls start early)
            uT32 = state.tile([128, M], F32)
            uTb = ustate.tile([128, M], BF16, tag="uTb")
            unat = ustate.tile([128, MB, 128], BF16, tag="unat")
            vb = state.tile([128, D2], BF16)
            vT = state.tile([128, NB, 128], BF16)
            nc.sync.dma_start(uT32[:], uT32_d[:])
            nc.scalar.copy(uTb[:], uT32[:])
            nc.sync.dma_start(unat[:], unat_d[:])
            nc.sync.dma_start(vb[:], vb_d[:])
            nc.sync.dma_start(vT[:], vT_d[:])

            self_state = {}
            mxT = state.tile([128, NB, M], BF16)
            mskT = state.tile([128, NB, M], BF16)
            for c in range(8):
                s = slice(c * (NB // 8), (c + 1) * (NB // 8))
                nc.sync.dma_start(mskT[:, s], mskT_d[:, s])
                nc.sync.dma_start(mxT[:, s], mxT_d[:, s])

            warm_in = dram.tile([128, 16], BF16, tag="warm_in", name="warm_in")
            warm_out = dram.tile([128, 16], BF16, addr_space="Shared",
                                 tag="warm_out", name="warm_out")
            warm_sb = rot.tile([128, 16], BF16, tag="warm")
            nc.vector.tensor_copy(warm_sb[:], vT[:, 0, 0:16])
            nc.sync.dma_start(warm_in[:], warm_sb[:])
            nc.gpsimd.collective_compute(
                "AllReduce", mybir.AluOpType.add,
                replica_groups=[list(range(N_CORES))],
                ins=[warm_in.opt()], outs=[warm_out.opt()],
            )

            def e_group(g, e_nat, ud_ps):
                """M1 pair + cast + mask ops for group g (cols g*256..)."""
                ps = psP.tile([128, 1024], F32, tag="pT")
                for h in range(2):
                    nb = 2 * g + h
                    nc.tensor.matmul(
                        ps[:, h * 512:(h + 1) * 512],
                        vb[:, nb * 128:(nb + 1) * 128], uTb[:],
                        start=True, stop=True, skip_group_check=True,
                    )
                pTb = rot.tile([128, 1024], BF16, tag="pTb")
                nc.scalar.copy(pTb[:], ps[:])
                q = rot.tile([128, 1024], BF16, tag="q")
                nc.vector.tensor_tensor(
                    q[:], mskT[:, 2 * g:2 * g + 2], pTb[:],
                    mybir.AluOpType.mult)
                if g % 2 == 0:
                    eT = etp.tile([128, 2048], BF16, tag="eT",
                                  name=f"eT_{g}")
                else:
                    eT = self_state["eT"]
                half = (g % 2) * 1024
                nc.vector.tensor_tensor(
                    eT[:, half:half + 1024], mxT[:, 2 * g:2 * g + 2], q[:],
                    mybir.AluOpType.subtract)
                if g % 2 == 1:
                    nc.sync.dma_start_transpose(
                        e_nat[:, 2 * (g - 1):2 * (g - 1) + 4, :, :], eT[:])
                self_state["eT"] = eT
                return eT

            def m2_group(g, eT, ud_ps):
                base = (g % 2) * 1024
                for h in range(2):
                    nb = 2 * g + h
                    nc.tensor.matmul(
                        ud_ps[:], vT[:, nb],
                        eT[:, base + h * 512:base + (h + 1) * 512],
                        start=(nb == 0), stop=(nb == NB - 1),
                        skip_group_check=True,
                    )

            for it in range(ITERS):
                e_nat = enatp.tile([128, NB, MB, 128], BF16, tag="e_nat")
                ud_ps = psU.tile([128, M], F32, tag="ud")

                # M1/e stream with M2 chasing one group behind (keeps PE
                # from stalling on eT while still bounding eT liveness).
                self_state.clear()
                ets = {}
                for g in range(NG):
                    ets[g] = e_group(g, e_nat, ud_ps)
                    if g >= 1:
                        m2_group(g - 1, ets.pop(g - 1), ud_ps)
                m2_group(NG - 1, ets.pop(NG - 1), ud_ps)

                # ---- U update (U_T += Udelta_T), refresh bf16 + U_nat
                nc.vector.tensor_tensor(
                    uT32[:], uT32[:], ud_ps[:], mybir.AluOpType.add)
                uTb = ustate.tile([128, M], BF16, tag="uTb")
                nc.scalar.copy(uTb[:], uT32[:])
                unat = ustate.tile([128, MB, 128], BF16, tag="unat")
                nc.sync.dma_start_transpose(unat[:], uTb[:])

                # ---- M3: G = U_new^T E in 4 slabs of 1024 cols, then one
                # 1MB bf16 AllReduce for the V update.
                cc_in = dram.tile([128, D2], BF16, tag="cc_in",
                                  name=f"cc_in_{it}")
                cc_out = dram.tile([128, D2], BF16, addr_space="Shared",
                                   tag="cc_out", name=f"cc_out_{it}")
                g_sb = goutp.tile([128, D2], BF16, tag="g")
                for gg in range(4):
                    ps = psP.tile([128, 1024], F32, tag="pT")
                    for h in range(2):
                        ch = 2 * gg + h
                        for mb in range(MB):
                            nc.tensor.matmul(
                                ps[:, h * 512:(h + 1) * 512], unat[:, mb],
                                e_nat[:, 4 * ch:4 * ch + 4, mb, :],
                                start=(mb == 0), stop=(mb == MB - 1),
                                skip_group_check=True,
                            )
                    nc.scalar.copy(g_sb[:, gg * 1024:(gg + 1) * 1024], ps[:])
                    nc.sync.dma_start(
                        cc_in[:, gg * 1024:(gg + 1) * 1024],
                        g_sb[:, gg * 1024:(gg + 1) * 1024])
                nc.gpsimd.collective_compute(
                    "AllReduce",
                    mybir.AluOpType.add,
                    replica_groups=[list(range(N_CORES))],
                    ins=[cc_in.opt()],
                    outs=[cc_out.opt()],
                )

                # ---- V += dV (nat + transposed copies), in column slices so
                # the next iteration's first groups can start early. On the
                # last iteration the output phase consumes dV directly
                # (out = U3 V2 + U3 dV3) so V itself is not updated.
                dv = dvp.tile([128, D2], BF16, tag="dv")
                nc.sync.dma_start(dv[:], cc_out[:])
                if it < ITERS - 1:
                    dvT = dvp.tile([128, NB, 128], BF16, tag="dvT")
                    nc.sync.dma_start_transpose(dvT[:], cc_out[:])
                    for sl in range(4):
                        cols = slice(sl * 1024, (sl + 1) * 1024)
                        nc.vector.tensor_tensor(
                            vb[:, cols], vb[:, cols], dv[:, cols],
                            mybir.AluOpType.add)
                        nc.vector.tensor_tensor(
                            vT[:, sl * 8:(sl + 1) * 8],
                            vT[:, sl * 8:(sl + 1) * 8],
                            dvT[:, sl * 8:(sl + 1) * 8], mybir.AluOpType.add)

            # ---- output: out = U3 V2 + U3 dV3 (A-part matmuls run during
            # the last AllReduce; C-part accumulates once dV arrives)
            for mb in range(MB):
                for cg in range(4):
                    ps = psP.tile([128, 1024], F32, tag="pT")
                    for k in range(2):
                        ch = cg * 2 + k
                        nc.tensor.matmul(
                            ps[:, k * 512:(k + 1) * 512],
                            uTb[:, mb * 128:(mb + 1) * 128],
                            vb[:, ch * 512:(ch + 1) * 512],
                            start=True, stop=True, skip_group_check=True,
                        )
                    for k in range(2):
                        nc.tensor.matmul(
                            ps[:, k * 512:(k + 1) * 512],
                            uTb[:, mb * 128:(mb + 1) * 128],
                            dv[:, cg * 1024 + k * 512:cg * 1024 + (k + 1) * 512],
                            start=False, stop=True, skip_group_check=True,
                        )
                    o_sb = ostage.tile([128, 1024], F32, tag="o",
                                       name=f"o_sb_{mb}_{cg}")
                    if (mb + cg) % 2 == 0:
                        nc.scalar.copy(o_sb[:], ps[:])
                    else:
                        nc.vector.tensor_copy(o_sb[:], ps[:])
                    nc.sync.dma_start(
                        out_d[mb * 128:(mb + 1) * 128,
                              cg * 1024:(cg + 1) * 1024],
                        o_sb[:])

    nc.compile()
    return nc


def _prep_inputs(x, mask, U, V):
    x = np.ascontiguousarray(np.asarray(x, dtype=np.float32))
    mask = np.ascontiguousarray(np.asarray(mask, dtype=np.float32))
    U = np.ascontiguousarray(np.asarray(U, dtype=np.float32))
    V = np.ascontiguousarray(np.asarray(V, dtype=np.float32))
    mx = mask * x

    vb = V.astype(bf16)                                    # [128, D2]
    vT = np.ascontiguousarray(
        V.T.reshape(NB, 128, 128).transpose(1, 0, 2)).astype(bf16)

    in_maps = []
    for i in range(N_CORES):
        rows = slice(i * M, (i + 1) * M)
        mxT = np.ascontiguousarray(
            mx[rows].T.reshape(NB, 128, M).transpose(1, 0, 2)).astype(bf16)
        mskT = np.ascontiguousarray(
            mask[rows].T.reshape(NB, 128, M).transpose(1, 0, 2)).astype(bf16)
        uT32 = np.ascontiguousarray(U[rows].T)             # [128, M] f32
        unat = np.ascontiguousarray(
            U[rows].reshape(MB, 128, 128).transpose(1, 0, 2)).astype(bf16)
        in_maps.append({
            "mxT": mxT, "mskT": mskT, "uT32": uT32, "unat": unat,
            "vb": vb, "vT": vT,
        })
    return in_maps


def kernel(x, mask, U, V, _trace=False):
    if "nc" not in _cache:
        _cache["nc"] = _build()
    nc = _cache["nc"]
    in_maps = _prep_inputs(x, mask, U, V)
    res = run_bass_kernel_spmd(
        nc, in_maps, core_ids=list(range(N_CORES)), trace=_trace)
    _cache["last_result"] = res
    out = np.concatenate([res.results[i]["out"] for i in range(N_CORES)],
                         axis=0)
    return out.astype(np.float32)


# revision 16
# speedup vs baseline: 1.5202x; 1.5202x over previous
"""LRMC (masked low-rank matrix completion) kernel for 8 trn2 NeuronCores.

Algorithm (3 iterations):
    E = mask * (x - U V)        [d1, d2]
    U = U + E V^T               [d1, r]
    V = V + U^T E               [r, d2]   (updated U; V-update all-reduced)
out = U V.

Sharding: rows of x/mask/U over 8 cores (512 rows each); V replicated;
U^T E contribution all-reduced (bf16, split into two column halves so the
next iteration's column groups can start while the second half is in
flight) each iteration.

On-chip layouts (per core, m = 512 local rows, r = 128, d2 = 4096):
    mxT, maskT : [128p, 32nb, 512m]  bf16   (transposed masked-x / mask)
    e_T groups : [128p(d2), 1024]    bf16   (rotating, 2 nb blocks each)
    e_nat      : [128p(m), 4mb, 4096d2] bf16  (via DMA xbar transpose)
    U stored as U_T f32/bf16 [128r, 512m] + U_nat bf16 [128p(m), 4mb, 128r]
    V stored as V_nat bf16 [128r, 4096d2] + V_T bf16 [128p(d2), 32nb, 128r]

Matmuls (bf16 operands, fp32 PSUM):
    P_T[nb]   = matmul(lhsT=V_nat[:, nb*128:+128], rhs=U_T)          (M1)
    Udelta_T += matmul(lhsT=V_T[:, nb], rhs=e_T[nb])                 (M2)
    G[chunk] += matmul(lhsT=U_nat[:, mb], rhs=e_nat[:, mb, chunk])   (M3)
"""

import numpy as np
import ml_dtypes

import concourse.bass as bass
import concourse.tile as tile
from concourse import bacc, mybir
from concourse.bass_utils import run_bass_kernel_spmd

D1, D2, RANK, ITERS = 4096, 4096, 128, 3
N_CORES = 8
M = D1 // N_CORES          # 512 rows per core
MB = M // 128              # 4 row blocks
NB = D2 // 128             # 32 d2 blocks
NG = 16                    # groups of 2 nb blocks (1024 cols of P_T slab)
HALF = D2 // 2             # 2048
N_TICKS = 56               # PE keep-warm matmuls per AllReduce window

BF16 = mybir.dt.bfloat16
F32 = mybir.dt.float32
bf16 = ml_dtypes.bfloat16

_cache = {}


def _build():
    nc = bacc.Bacc("TRN2", target_bir_lowering=False, debug=False,
                   num_devices=N_CORES)

    mxT_d = nc.dram_tensor("mxT", [128, NB, M], BF16, kind="ExternalInput")
    mskT_d = nc.dram_tensor("mskT", [128, NB, M], BF16, kind="ExternalInput")
    uT32_d = nc.dram_tensor("uT32", [128, M], F32, kind="ExternalInput")
    unat_d = nc.dram_tensor("unat", [128, MB, 128], BF16, kind="ExternalInput")
    vb_d = nc.dram_tensor("vb", [128, D2], BF16, kind="ExternalInput")
    vT_d = nc.dram_tensor("vT", [128, NB, 128], BF16, kind="ExternalInput")
    out_d = nc.dram_tensor("out", [M, D2], BF16, kind="ExternalOutput")

    with tile.TileContext(nc) as tc:
        with (
            tc.tile_pool(name="state", bufs=1) as state,
            tc.tile_pool(name="ustate", bufs=2) as ustate,
            tc.tile_pool(name="enat", bufs=1) as enatp,
            tc.tile_pool(name="rot", bufs=3) as rot,
            tc.tile_pool(name="et", bufs=4) as etp,
            tc.tile_pool(name="gout", bufs=1) as goutp,
            tc.tile_pool(name="dvp", bufs=1) as dvp,
            tc.tile_pool(name="ostage", bufs=3) as ostage,
            tc.tile_pool(name="psP", bufs=3, space="PSUM") as psP,
            tc.tile_pool(name="psU", bufs=1, space="PSUM") as psU,
            tc.tile_pool(name="dram", bufs=2, space="DRAM") as dram,
        ):
            # ---- load state (small tensors first so matmuls start early)
            uT32 = state.tile([128, M], F32)
            uTb = ustate.tile([128, M], BF16, tag="uTb")
            unat = ustate.tile([128, MB, 128], BF16, tag="unat")
            vb = state.tile([128, D2], BF16)
            vT = state.tile([128, NB, 128], BF16)
            nc.sync.dma_start(uT32[:], uT32_d[:])
            nc.scalar.copy(uTb[:], uT32[:])
            nc.sync.dma_start(unat[:], unat_d[:])
            nc.sync.dma_start(vb[:], vb_d[:])
            nc.sync.dma_start(vT[:], vT_d[:])

            self_state = {}
            mxT = state.tile([128, NB, M], BF16)
            mskT = state.tile([128, NB, M], BF16)
            for c in range(8):
                s = slice(c * (NB // 8), (c + 1) * (NB // 8))
                nc.sync.dma_start(mskT[:, s], mskT_d[:, s])
                nc.sync.dma_start(mxT[:, s], mxT_d[:, s])

            warm_in = dram.tile([128, 16], BF16, tag="warm_in", name="warm_in")
            warm_out = dram.tile([128, 16], BF16, addr_space="Shared",
                                 tag="warm_out", name="warm_out")
            warm_sb = rot.tile([128, 16], BF16, tag="warm")
            nc.vector.tensor_copy(warm_sb[:], vT[:, 0, 0:16])
            nc.sync.dma_start(warm_in[:], warm_sb[:])
            nc.gpsimd.collective_compute(
                "AllReduce", mybir.AluOpType.add,
                replica_groups=[list(range(N_CORES))],
                ins=[warm_in.opt()], outs=[warm_out.opt()],
            )

            def e_group(g, e_nat, ud_ps):
                """M1 pair + cast + mask ops for group g (cols g*256..)."""
                ps = psP.tile([128, 1024], F32, tag="pT")
                for h in range(2):
                    nb = 2 * g + h
                    nc.tensor.matmul(
                        ps[:, h * 512:(h + 1) * 512],
                        vb[:, nb * 128:(nb + 1) * 128], uTb[:],
                        start=True, stop=True, skip_group_check=True,
                    )
                if g % 2 == 0:
                    pTb = rot.tile([128, 2048], BF16, tag="pTb",
                                   name=f"pTb_{g}")
                    self_state["pTb"] = pTb
                else:
                    pTb = self_state["pTb"]
                half = (g % 2) * 1024
                nc.scalar.copy(pTb[:, half:half + 1024], ps[:])
                if g % 2 == 1:
                    q = rot.tile([128, 2048], BF16, tag="q", name=f"q_{g}")
                    nc.vector.tensor_tensor(
                        q[:], mskT[:, 2 * g - 2:2 * g + 2], pTb[:],
                        mybir.AluOpType.mult)
                    eT = etp.tile([128, 2048], BF16, tag="eT",
                                  name=f"eT_{g}")
                    nc.vector.tensor_tensor(
                        eT[:], mxT[:, 2 * g - 2:2 * g + 2], q[:],
                        mybir.AluOpType.subtract)
                    nc.sync.dma_start_transpose(
                        e_nat[:, 2 * (g - 1):2 * (g - 1) + 4, :, :], eT[:])
                    self_state["eT"] = eT
                    return eT
                return None

            def m2_pair(gpair, eT, ud_ps):
                for j in range(4):
                    nb = 4 * gpair + j
                    nc.tensor.matmul(
                        ud_ps[:], vT[:, nb],
                        eT[:, j * 512:(j + 1) * 512],
                        start=(nb == 0), stop=(nb == NB - 1),
                        skip_group_check=True,
                    )

            for it in range(ITERS):
                e_nat = enatp.tile([128, NB, MB, 128], BF16, tag="e_nat")
                ud_ps = psU.tile([128, M], F32, tag="ud")

                # M1/e stream with M2 chasing one group behind (keeps PE
                # from stalling on eT while still bounding eT liveness).
                self_state.clear()
                prev = None
                for g in range(NG):
                    r = e_group(g, e_nat, ud_ps)
                    if r is not None:
                        if prev is not None:
                            m2_pair((g - 2) // 2, prev, ud_ps)
                        prev = r
                m2_pair((NG - 2) // 2, prev, ud_ps)

                # ---- U update (U_T += Udelta_T), refresh bf16 + U_nat
                nc.vector.tensor_tensor(
                    uT32[:], uT32[:], ud_ps[:], mybir.AluOpType.add)
                uTb = ustate.tile([128, M], BF16, tag="uTb")
                nc.scalar.copy(uTb[:], uT32[:])
                unat = ustate.tile([128, MB, 128], BF16, tag="unat")
                nc.sync.dma_start_transpose(unat[:], uTb[:])

                # ---- M3: G = U_new^T E in 4 slabs of 1024 cols, then one
                # 1MB bf16 AllReduce for the V update.
                cc_in = dram.tile([128, D2], BF16, tag="cc_in",
                                  name=f"cc_in_{it}")
                cc_out = dram.tile([128, D2], BF16, addr_space="Shared",
                                   tag="cc_out", name=f"cc_out_{it}")
                g_sb = goutp.tile([128, D2], BF16, tag="g")
                for gg in range(4):
                    ps = psP.tile([128, 1024], F32, tag="pT")
                    for h in range(2):
                        ch = 2 * gg + h
                        for mb in range(MB):
                            nc.tensor.matmul(
                                ps[:, h * 512:(h + 1) * 512], unat[:, mb],
                                e_nat[:, 4 * ch:4 * ch + 4, mb, :],
                                start=(mb == 0), stop=(mb == MB - 1),
                                skip_group_check=True,
                            )
                    nc.scalar.copy(g_sb[:, gg * 1024:(gg + 1) * 1024], ps[:])
                    nc.sync.dma_start(
                        cc_in[:, gg * 1024:(gg + 1) * 1024],
                        g_sb[:, gg * 1024:(gg + 1) * 1024])
                nc.gpsimd.collective_compute(
                    "AllReduce",
                    mybir.AluOpType.add,
                    replica_groups=[list(range(N_CORES))],
                    ins=[cc_in.opt()],
                    outs=[cc_out.opt()],
                )

                # ---- V += dV (nat + transposed copies), in column slices so
                # the next iteration's first groups can start early. The last
                # iteration needs no vT update (only vb for the output).
                dv = dvp.tile([128, D2], BF16, tag="dv")
                nc.sync.dma_start(dv[:], cc_out[:])
                if it < ITERS - 1:
                    dvT = dvp.tile([128, NB, 128], BF16, tag="dvT")
                    nc.sync.dma_start_transpose(dvT[:], cc_out[:])
                for sl in range(4):
                    cols = slice(sl * 1024, (sl + 1) * 1024)
                    nc.vector.tensor_tensor(
                        vb[:, cols], vb[:, cols], dv[:, cols],
                        mybir.AluOpType.add)
                    if it < ITERS - 1:
                        nc.vector.tensor_tensor(
                            vT[:, sl * 8:(sl + 1) * 8],
                            vT[:, sl * 8:(sl + 1) * 8],
                            dvT[:, sl * 8:(sl + 1) * 8], mybir.AluOpType.add)

            # ---- output: P = U3 V3 natural orientation, bf16 (host
            # widens to fp32); casts alternate ACT/DVE, DMA per slab.
            for mb in range(MB):
                for cg in range(4):
                    ps = psP.tile([128, 1024], F32, tag="pT")
                    for k in range(2):
                        ch = cg * 2 + k
                        nc.tensor.matmul(
                            ps[:, k * 512:(k + 1) * 512],
                            uTb[:, mb * 128:(mb + 1) * 128],
                            vb[:, ch * 512:(ch + 1) * 512],
                            start=True, stop=True, skip_group_check=True,
                        )
                    o_sb = ostage.tile([128, 1024], BF16, tag="o",
                                       name=f"o_sb_{mb}_{cg}")
                    if (mb + cg) % 2 == 0:
                        nc.scalar.copy(o_sb[:], ps[:])
                    else:
                        nc.vector.tensor_copy(o_sb[:], ps[:])
                    nc.sync.dma_start(
                        out_d[mb * 128:(mb + 1) * 128,
                              cg * 1024:(cg + 1) * 1024],
                        o_sb[:])

    nc.compile()
    return nc


def _prep_inputs(x, mask, U, V):
    x = np.ascontiguousarray(np.asarray(x, dtype=np.float32))
    mask = np.ascontiguousarray(np.asarray(mask, dtype=np.float32))
    U = np.ascontiguousarray(np.asarray(U, dtype=np.float32))
    V = np.ascontiguousarray(np.asarray(V, dtype=np.float32))
    mx = mask * x

    vb = V.astype(bf16)                                    # [128, D2]
    vT = np.ascontiguousarray(
        V.T.reshape(NB, 128, 128).transpose(1, 0, 2)).astype(bf16)

    in_maps = []
    for i in range(N_CORES):
        rows = slice(i * M, (i + 1) * M)
        mxT = np.ascontiguousarray(
            mx[rows].T.reshape(NB, 128, M).transpose(1, 0, 2)).astype(bf16)
        mskT = np.ascontiguousarray(
            mask[rows].T.reshape(NB, 128, M).transpose(1, 0, 2)).astype(bf16)
        uT32 = np.ascontiguousarray(U[rows].T)             # [128, M] f32
        unat = np.ascontiguousarray(
            U[rows].reshape(MB, 128, 128).transpose(1, 0, 2)).astype(bf16)
        in_maps.append({
            "mxT": mxT, "mskT": mskT, "uT32": uT32, "unat": unat,
            "vb": vb, "vT": vT,
        })
    return in_maps


def kernel(x, mask, U, V, _trace=False):
    if "nc" not in _cache:
        _cache["nc"] = _build()
    nc = _cache["nc"]
    in_maps = _prep_inputs(x, mask, U, V)
    res = run_bass_kernel_spmd(
        nc, in_maps, core_ids=list(range(N_CORES)), trace=_trace)
    _cache["last_result"] = res
    out = np.concatenate([res.results[i]["out"] for i in range(N_CORES)],
                         axis=0)
    return out.astype(np.float32)


# revision 17
# speedup vs baseline: 1.6321x; 1.0736x over previous
"""LRMC (masked low-rank matrix completion) kernel for 8 trn2 NeuronCores.

Algorithm (3 iterations):
    E = mask * (x - U V)        [d1, d2]
    U = U + E V^T               [d1, r]
    V = V + U^T E               [r, d2]   (updated U; V-update all-reduced)
out = U V.

Sharding: rows of x/mask/U over 8 cores (512 rows each); V replicated;
U^T E contribution all-reduced (bf16, split into two column halves so the
next iteration's column groups can start while the second half is in
flight) each iteration.

On-chip layouts (per core, m = 512 local rows, r = 128, d2 = 4096):
    mxT, maskT : [128p, 32nb, 512m]  bf16   (transposed masked-x / mask)
    e_T groups : [128p(d2), 1024]    bf16   (rotating, 2 nb blocks each)
    e_nat      : [128p(m), 4mb, 4096d2] bf16  (via DMA xbar transpose)
    U stored as U_T f32/bf16 [128r, 512m] + U_nat bf16 [128p(m), 4mb, 128r]
    V stored as V_nat bf16 [128r, 4096d2] + V_T bf16 [128p(d2), 32nb, 128r]

Matmuls (bf16 operands, fp32 PSUM):
    P_T[nb]   = matmul(lhsT=V_nat[:, nb*128:+128], rhs=U_T)          (M1)
    Udelta_T += matmul(lhsT=V_T[:, nb], rhs=e_T[nb])                 (M2)
    G[chunk] += matmul(lhsT=U_nat[:, mb], rhs=e_nat[:, mb, chunk])   (M3)
"""

import numpy as np
import ml_dtypes

import concourse.bass as bass
import concourse.tile as tile
from concourse import bacc, mybir
from concourse.bass_utils import run_bass_kernel_spmd

D1, D2, RANK, ITERS = 4096, 4096, 128, 3
N_CORES = 8
M = D1 // N_CORES          # 512 rows per core
MB = M // 128              # 4 row blocks
NB = D2 // 128             # 32 d2 blocks
NG = 16                    # groups of 2 nb blocks (1024 cols of P_T slab)
HALF = D2 // 2             # 2048
N_TICKS = 56               # PE keep-warm matmuls per AllReduce window

BF16 = mybir.dt.bfloat16
F32 = mybir.dt.float32
bf16 = ml_dtypes.bfloat16

_cache = {}


def _build():
    nc = bacc.Bacc("TRN2", target_bir_lowering=False, debug=False,
                   num_devices=N_CORES)

    mxT_d = nc.dram_tensor("mxT", [128, NB, M], BF16, kind="ExternalInput")
    mskT_d = nc.dram_tensor("mskT", [128, NB, M], BF16, kind="ExternalInput")
    uT32_d = nc.dram_tensor("uT32", [128, M], F32, kind="ExternalInput")
    unat_d = nc.dram_tensor("unat", [128, MB, 128], BF16, kind="ExternalInput")
    vb_d = nc.dram_tensor("vb", [128, D2], BF16, kind="ExternalInput")
    vT_d = nc.dram_tensor("vT", [128, NB, 128], BF16, kind="ExternalInput")
    out_d = nc.dram_tensor("out", [M, D2], BF16, kind="ExternalOutput")

    with tile.TileContext(nc) as tc:
        with (
            tc.tile_pool(name="state", bufs=1) as state,
            tc.tile_pool(name="ustate", bufs=2) as ustate,
            tc.tile_pool(name="enat", bufs=1) as enatp,
            tc.tile_pool(name="rot", bufs=3) as rot,
            tc.tile_pool(name="et", bufs=4) as etp,
            tc.tile_pool(name="gout", bufs=1) as goutp,
            tc.tile_pool(name="dvp", bufs=1) as dvp,
            tc.tile_pool(name="ostage", bufs=3) as ostage,
            tc.tile_pool(name="psP", bufs=3, space="PSUM") as psP,
            tc.tile_pool(name="psU", bufs=1, space="PSUM") as psU,
            tc.tile_pool(name="dram", bufs=2, space="DRAM") as dram,
        ):
            # ---- load state (small tensors first so matmuls start early)
            uT32 = state.tile([128, M], F32)
            uTb = ustate.tile([128, M], BF16, tag="uTb")
            unat = ustate.tile([128, MB, 128], BF16, tag="unat")
            vb = state.tile([128, D2], BF16)
            vT = state.tile([128, NB, 128], BF16)
            nc.sync.dma_start(uT32[:], uT32_d[:])
            nc.scalar.copy(uTb[:], uT32[:])
            nc.sync.dma_start(unat[:], unat_d[:])
            nc.sync.dma_start(vb[:], vb_d[:])
            nc.sync.dma_start(vT[:], vT_d[:])

            self_state = {}
            mxT = state.tile([128, NB, M], BF16)
            mskT = state.tile([128, NB, M], BF16)
            for c in range(8):
                s = slice(c * (NB // 8), (c + 1) * (NB // 8))
                nc.sync.dma_start(mskT[:, s], mskT_d[:, s])
                nc.sync.dma_start(mxT[:, s], mxT_d[:, s])

            def e_group(g, e_nat, ud_ps):
                """M1 pair + cast + mask ops for group g (cols g*256..)."""
                ps = psP.tile([128, 1024], F32, tag="pT")
                for h in range(2):
                    nb = 2 * g + h
                    nc.tensor.matmul(
                        ps[:, h * 512:(h + 1) * 512],
                        vb[:, nb * 128:(nb + 1) * 128], uTb[:],
                        start=True, stop=True, skip_group_check=True,
                    )
                if g % 2 == 0:
                    pTb = rot.tile([128, 2048], BF16, tag="pTb",
                                   name=f"pTb_{g}")
                    self_state["pTb"] = pTb
                else:
                    pTb = self_state["pTb"]
                half = (g % 2) * 1024
                nc.scalar.copy(pTb[:, half:half + 1024], ps[:])
                if g % 2 == 1:
                    q = rot.tile([128, 2048], BF16, tag="q", name=f"q_{g}")
                    nc.vector.tensor_tensor(
                        q[:], mskT[:, 2 * g - 2:2 * g + 2], pTb[:],
                        mybir.AluOpType.mult)
                    eT = etp.tile([128, 2048], BF16, tag="eT",
                                  name=f"eT_{g}")
                    nc.vector.tensor_tensor(
                        eT[:], mxT[:, 2 * g - 2:2 * g + 2], q[:],
                        mybir.AluOpType.subtract)
                    nc.sync.dma_start_transpose(
                        e_nat[:, 2 * (g - 1):2 * (g - 1) + 4, :, :], eT[:])
                    self_state["eT"] = eT
                    return eT
                return None

            def m2_pair(gpair, eT, ud_ps):
                for j in range(4):
                    nb = 4 * gpair + j
                    nc.tensor.matmul(
                        ud_ps[:], vT[:, nb],
                        eT[:, j * 512:(j + 1) * 512],
                        start=(nb == 0), stop=(nb == NB - 1),
                        skip_group_check=True,
                    )

            for it in range(ITERS):
                e_nat = enatp.tile([128, NB, MB, 128], BF16, tag="e_nat")
                ud_ps = psU.tile([128, M], F32, tag="ud")

                # M1/e stream with M2 chasing one group behind (keeps PE
                # from stalling on eT while still bounding eT liveness).
                self_state.clear()
                prev = None
                for g in range(NG):
                    r = e_group(g, e_nat, ud_ps)
                    if r is not None:
                        if prev is not None:
                            m2_pair((g - 2) // 2, prev, ud_ps)
                        prev = r
                m2_pair((NG - 2) // 2, prev, ud_ps)

                # ---- U update (U_T += Udelta_T), refresh bf16 + U_nat
                nc.vector.tensor_tensor(
                    uT32[:], uT32[:], ud_ps[:], mybir.AluOpType.add)
                uTb = ustate.tile([128, M], BF16, tag="uTb")
                nc.scalar.copy(uTb[:], uT32[:])
                unat = ustate.tile([128, MB, 128], BF16, tag="unat")
                nc.sync.dma_start_transpose(unat[:], uTb[:])

                # ---- M3: G = U_new^T E in 4 slabs of 1024 cols, then one
                # 1MB bf16 AllReduce for the V update.
                cc_in = dram.tile([128, D2], BF16, tag="cc_in",
                                  name=f"cc_in_{it}")
                cc_out = dram.tile([128, D2], BF16, addr_space="Shared",
                                   tag="cc_out", name=f"cc_out_{it}")
                g_sb = goutp.tile([128, D2], BF16, tag="g")
                for gg in range(4):
                    ps = psP.tile([128, 1024], F32, tag="pT")
                    for h in range(2):
                        ch = 2 * gg + h
                        for mb in range(MB):
                            nc.tensor.matmul(
                                ps[:, h * 512:(h + 1) * 512], unat[:, mb],
                                e_nat[:, 4 * ch:4 * ch + 4, mb, :],
                                start=(mb == 0), stop=(mb == MB - 1),
                                skip_group_check=True,
                            )
                    nc.scalar.copy(g_sb[:, gg * 1024:(gg + 1) * 1024], ps[:])
                    nc.sync.dma_start(
                        cc_in[:, gg * 1024:(gg + 1) * 1024],
                        g_sb[:, gg * 1024:(gg + 1) * 1024])
                nc.gpsimd.collective_compute(
                    "AllReduce",
                    mybir.AluOpType.add,
                    replica_groups=[list(range(N_CORES))],
                    ins=[cc_in.opt()],
                    outs=[cc_out.opt()],
                )

                # ---- V += dV (nat + transposed copies), in column slices so
                # the next iteration's first groups can start early. The last
                # iteration needs no vT update (only vb for the output).
                dv = dvp.tile([128, D2], BF16, tag="dv")
                nc.sync.dma_start(dv[:], cc_out[:])
                if it < ITERS - 1:
                    dvT = dvp.tile([128, NB, 128], BF16, tag="dvT")
                    nc.sync.dma_start_transpose(dvT[:], cc_out[:])
                for sl in range(4):
                    cols = slice(sl * 1024, (sl + 1) * 1024)
                    nc.vector.tensor_tensor(
                        vb[:, cols], vb[:, cols], dv[:, cols],
                        mybir.AluOpType.add)
                    if it < ITERS - 1:
                        nc.vector.tensor_tensor(
                            vT[:, sl * 8:(sl + 1) * 8],
                            vT[:, sl * 8:(sl + 1) * 8],
                            dvT[:, sl * 8:(sl + 1) * 8], mybir.AluOpType.add)

            # ---- output: P = U3 V3 natural orientation, bf16 (host
            # widens to fp32); casts alternate ACT/DVE, DMA per slab.
            for mb in range(MB):
                for cg in range(4):
                    ps = psP.tile([128, 1024], F32, tag="pT")
                    for k in range(2):
                        ch = cg * 2 + k
                        nc.tensor.matmul(
                            ps[:, k * 512:(k + 1) * 512],
                            uTb[:, mb * 128:(mb + 1) * 128],
                            vb[:, ch * 512:(ch + 1) * 512],
                            start=True, stop=True, skip_group_check=True,
                        )
                    o_sb = ostage.tile([128, 1024], BF16, tag="o",
                                       name=f"o_sb_{mb}_{cg}")
                    if (mb + cg) % 2 == 0:
                        nc.scalar.copy(o_sb[:], ps[:])
                    else:
                        nc.vector.tensor_copy(o_sb[:], ps[:])
                    nc.sync.dma_start(
                        out_d[mb * 128:(mb + 1) * 128,
                              cg * 1024:(cg + 1) * 1024],
                        o_sb[:])

    nc.compile()
    return nc


def _prep_inputs(x, mask, U, V):
    x = np.ascontiguousarray(np.asarray(x, dtype=np.float32))
    mask = np.ascontiguousarray(np.asarray(mask, dtype=np.float32))
    U = np.ascontiguousarray(np.asarray(U, dtype=np.float32))
    V = np.ascontiguousarray(np.asarray(V, dtype=np.float32))
    mx = mask * x

    vb = V.astype(bf16)                                    # [128, D2]
    vT = np.ascontiguousarray(
        V.T.reshape(NB, 128, 128).transpose(1, 0, 2)).astype(bf16)

    in_maps = []
    for i in range(N_CORES):
        rows = slice(i * M, (i + 1) * M)
        mxT = np.ascontiguousarray(
            mx[rows].T.reshape(NB, 128, M).transpose(1, 0, 2)).astype(bf16)
        mskT = np.ascontiguousarray(
            mask[rows].T.reshape(NB, 128, M).transpose(1, 0, 2)).astype(bf16)
        uT32 = np.ascontiguousarray(U[rows].T)             # [128, M] f32
        unat = np.ascontiguousarray(
            U[rows].reshape(MB, 128, 128).transpose(1, 0, 2)).astype(bf16)
        in_maps.append({
            "mxT": mxT, "mskT": mskT, "uT32": uT32, "unat": unat,
            "vb": vb, "vT": vT,
        })
    return in_maps


def kernel(x, mask, U, V, _trace=False):
    if "nc" not in _cache:
        _cache["nc"] = _build()
    nc = _cache["nc"]
    in_maps = _prep_inputs(x, mask, U, V)
    res = run_bass_kernel_spmd(
        nc, in_maps, core_ids=list(range(N_CORES)), trace=_trace)
    _cache["last_result"] = res
    out = np.concatenate([res.results[i]["out"] for i in range(N_CORES)],
                         axis=0)
    return out.astype(np.float32)
